# revision 20
# baseline (speedup 1.0000x reference)
"""HONU order-3 kernel for 8 TRN2 NeuronCores (v5).

Math: out[b] = sum_{i<=j<=k} w_ijk * xf_i * xf_j * xf_k,  xf = [1, x] (127 feats).

Restructuring: for each pair p=(i,j) (i<=j, 8128 pairs) let
    W[k, p] = w_ijk for k in [j,127)  (0 otherwise)
    Z[b, p] = sum_k W[k, p] * xf[b, k]          (dense matmul)
    out[b]  = sum_p Z[b, p] * xf_i(p)[b] * xf_j(p)[b]

The two per-pair factors are HOST-GATHERED index tensors (pure layout, no
arithmetic): XS[b, p] = xf[b, i(p)], XB[b, p] = xf[b, j(p)].  On-device, per
128-batch tile:
    zc = bf16(Z)            ACT copy, PSUM -> SBUF        (per 512 chunk)
    v1 = zc * XS            tensor_tensor, bf16 2x mode   (per 512 chunk)
    res = sum_p (v1 * XB)   scalar_tensor_tensor accum    (per tile; STT has
                            no DVE fast mode, but the fused accumulate beats
                            TT + 1x reduce)

Sharding: pair p -> core p%8 (1016 pairs/core, padded to 1024 columns; two
512-col matmul chunks per batch tile for the PSUM bank limit).  x is
replicated; each core returns a [256]-shaped partial that the host sums.

DMA: the per-core input set (~1.4MB) rides all three DMA channels (sync,
scalar/ACT, gpsimd SWDGE), split into ~128-256KB pieces ordered by first-use
time so compute never stalls on a late tensor.  res [128,2] is PE-transposed
to [2,128] so the output DMA is two contiguous 512B descriptors (a [128,x]
partition-strided store pays a multi-us receipt storm).
"""

import os
import numpy as np
import ml_dtypes

import concourse.bass as bass
import concourse.bacc as bacc
import concourse.tile as tile
import concourse.mybir as mybir
from concourse.bass_utils import run_bass_kernel_spmd

F32 = mybir.dt.float32
BF16 = mybir.dt.bfloat16
NPBF16 = ml_dtypes.bfloat16

P = 128
NF = 127              # features incl. bias
B = 256               # batch
NPAIR = 8128          # pairs (i<=j), i,j in [0,127)
NCORES = 8
NLOC = NPAIR // NCORES          # 1016 pairs per core
NCOL = 1024                     # padded columns
NCK = 512                       # matmul chunk (PSUM fp32 bank limit)

_CACHE = {}


def _build_nc():
    nc = bacc.Bacc("TRN2", target_bir_lowering=False, debug=False)
    xt = nc.dram_tensor("xt", [P, B], BF16, kind="ExternalInput")
    wds = [nc.dram_tensor(f"wd{ck}", [P, NCK], BF16, kind="ExternalInput")
           for ck in range(2)]
    xss = [[nc.dram_tensor(f"xs{bt}c{ck}", [P, NCK], BF16, kind="ExternalInput")
            for ck in range(2)] for bt in range(2)]
    xbs = [nc.dram_tensor(f"xb{bt}", [P, NCOL], BF16, kind="ExternalInput")
           for bt in range(2)]
    ident = nc.dram_tensor("ident", [P, P], F32, kind="ExternalInput")
    out = nc.dram_tensor("out", [2, P], F32, kind="ExternalOutput")

    with tile.TileContext(nc) as tc:
        with (
            tc.tile_pool(name="const", bufs=1) as cpool,
            tc.tile_pool(name="ps", bufs=1, space="PSUM") as ps,
        ):
            xt_t = cpool.tile([P, B], BF16, tag="xt")
            wd_t = [cpool.tile([P, NCK], BF16, tag=f"wd{ck}", name=f"wd{ck}_t")
                    for ck in range(2)]
            xs_t = [[cpool.tile([P, NCK], BF16, tag=f"xs{bt}c{ck}",
                                name=f"xs{bt}c{ck}_t") for ck in range(2)]
                    for bt in range(2)]
            xb_t = [cpool.tile([P, NCOL], BF16, tag=f"xb{bt}", name=f"xb{bt}_t")
                    for bt in range(2)]
            id_t = cpool.tile([P, P], F32, tag="ident")

            # --- input DMAs: 3 channels, ordered by first-use time ---
            nc.sync.dma_start(xt_t[:], xt[:])
            nc.scalar.dma_start(wd_t[1][:], wds[1][:])
            nc.sync.dma_start(wd_t[0][:], wds[0][:])
            nc.scalar.dma_start(xs_t[0][0][:], xss[0][0][:])
            nc.gpsimd.dma_start(xs_t[1][1][:], xss[1][1][:])
            nc.sync.dma_start(xb_t[0][:], xbs[0][:])
            nc.scalar.dma_start(xs_t[0][1][:], xss[0][1][:])
            nc.gpsimd.dma_start(id_t[:], ident[:])
            nc.sync.dma_start(xs_t[1][0][:], xss[1][0][:])
            nc.scalar.dma_start(xb_t[1][:], xbs[1][:])

            res_t = cpool.tile([P, 2], F32, tag="res")
            scr_t = cpool.tile([P, NCOL], BF16, tag="scr")
            for bt in range(2):
                v1 = cpool.tile([P, NCOL], BF16, tag=f"v1_{bt}", name=f"v1_{bt}")
                for ck in range(2):
                    z_ps = ps.tile([P, NCK], F32, tag=f"z{bt}{ck}",
                                   name=f"z{bt}{ck}_ps")
                    nc.tensor.matmul(
                        z_ps[:], xt_t[:, bt * P:(bt + 1) * P], wd_t[ck][:],
                        start=True, stop=True,
                    )
                    zc = cpool.tile([P, NCK], BF16, tag=f"zc{bt}{ck}",
                                    name=f"zc{bt}{ck}")
                    nc.scalar.copy(zc[:], z_ps[:])
                    nc.vector.tensor_tensor(
                        v1[:, ck * NCK:(ck + 1) * NCK],
                        zc[:],
                        xs_t[bt][ck][:],
                        mybir.AluOpType.mult,
                    )
                # fused multiply + row-accumulate via TensorScalarPtr
                nc.vector.scalar_tensor_tensor(
                    out=scr_t[:],
                    in0=v1[:],
                    scalar=1.0,
                    in1=xb_t[bt][:],
                    op0=mybir.AluOpType.mult,
                    op1=mybir.AluOpType.mult,
                    accum_out=res_t[:, bt:bt + 1],
                )

            # transpose [128,2] -> [2,128] on the PE so the output DMA is two
            # contiguous 512B descriptors instead of 128 8B ones
            tr_ps = ps.tile([2, P], F32, tag="tr")
            nc.tensor.transpose(tr_ps[:], res_t[:], id_t[:])
            stage = cpool.tile([2, P], F32, tag="stage")
            nc.vector.tensor_copy(stage[:], tr_ps[:])
            nc.sync.dma_start(out[:], stage[:])
    nc.compile()
    return nc


def _pair_maps():
    # lex-ordered pairs (i<=j): p = rsp2[i] + (j - i)
    i_of = np.repeat(np.arange(NF), NF - np.arange(NF))
    j_of = np.concatenate([np.arange(i, NF) for i in range(NF)])
    return i_of, j_of


def _prep_inputs(x, weights, comb_idx):
    """Host-side layout prep: gathers/scatters only, no arithmetic on x."""
    x = np.ascontiguousarray(np.asarray(x, dtype=np.float32))
    w = np.asarray(weights, dtype=np.float32).ravel()
    ci = np.asarray(comb_idx)
    i_, j_, k_ = (ci[:, 0].astype(np.int64), ci[:, 1].astype(np.int64),
                  ci[:, 2].astype(np.int64))

    xf = np.concatenate([np.ones((B, 1), np.float32), x], axis=1)  # [256,127]

    ar = np.arange(NF, dtype=np.int64)
    rsp2 = ar * NF - (ar * (ar - 1)) // 2
    p_of_c = rsp2[i_] + (j_ - i_)          # pair index of each triple
    W_all = np.zeros((P, NPAIR), np.float32)
    W_all[k_, p_of_c] = w

    i_of, j_of = _pair_maps()
    xf_d = xf.astype(NPBF16)
    xt = np.zeros((P, B), NPBF16)
    xt[:NF, :] = xf_d.T

    in_maps = []
    for c in range(NCORES):
        ps_ = np.arange(c, NPAIR, NCORES)
        wd = np.zeros((P, NCOL), NPBF16)
        wd[:, :NLOC] = W_all[:, ps_].astype(NPBF16)
        ic, jc = i_of[ps_], j_of[ps_]
        m = {"xt": xt, "ident": np.eye(P, dtype=np.float32)}
        for ck in range(2):
            m[f"wd{ck}"] = np.ascontiguousarray(wd[:, ck * NCK:(ck + 1) * NCK])
        for bt in range(2):
            xs = np.zeros((P, NCOL), NPBF16)
            xb = np.zeros((P, NCOL), NPBF16)
            xs[:, :NLOC] = xf_d[bt * P:(bt + 1) * P, :][:, ic]
            xb[:, :NLOC] = xf_d[bt * P:(bt + 1) * P, :][:, jc]
            for ck in range(2):
                m[f"xs{bt}c{ck}"] = np.ascontiguousarray(
                    xs[:, ck * NCK:(ck + 1) * NCK])
            m[f"xb{bt}"] = xb
        in_maps.append(m)
    return in_maps


def _get_nc():
    if "nc" not in _CACHE:
        _CACHE["nc"] = _build_nc()
    return _CACHE["nc"]


def run_spmd(x, weights, comb_idx, trace=False):
    nc = _get_nc()
    in_maps = _prep_inputs(x, weights, comb_idx)
    res = run_bass_kernel_spmd(nc, in_maps, list(range(NCORES)), trace=trace)
    acc = np.zeros(B, np.float64)
    for c in range(NCORES):
        acc += res.results[c]["out"].astype(np.float64).reshape(B)
    return acc[:, None].astype(np.float32), res


def kernel(x, weights, comb_idx):
    out, _ = run_spmd(x, weights, comb_idx, trace=False)
    return out


# revision 24
# speedup vs baseline: 1.2000x; 1.2000x over previous
"""HONU order-3 kernel for 8 TRN2 NeuronCores (v7).

Math: out[b] = sum_{i<=j<=k} w_ijk * xf_i * xf_j * xf_k,  xf = [1, x] (127 feats).

Restructuring: for each pair p=(i,j) (i<=j, 8128 pairs) let
    W[k, p] = w_ijk for k in [j,127)  (0 otherwise)
    Z[b, p] = sum_k W[k, p] * xf[b, k]          (dense matmul)
    out[b]  = sum_p Z[b, p] * xf_i(p)[b] * xf_j(p)[b]

The two per-pair factors are HOST-GATHERED index tensors (pure layout, no
arithmetic): XS[b, p] = xf[b, i(p)], XB[b, p] = xf[b, j(p)].  On-device, per
128-batch tile:
    zc = bf16(Z)            ACT copy, PSUM -> SBUF        (per 512 chunk)
    v1 = zc * XS            tensor_tensor, bf16 2x mode   (per 512 chunk)
    res = sum_p (v1 * XB)   scalar_tensor_tensor accum    (per tile; STT has
                            no DVE fast mode, but the fused accumulate beats
                            TT + 1x reduce)

Sharding: pair p -> core p%8 (1016 pairs/core, padded to 1024 columns; two
512-col matmul chunks per batch tile for the PSUM bank limit).  x is
replicated; each core returns a [256]-shaped partial that the host sums.

DMA: the per-core input set (~1.4MB) rides all three DMA channels (sync,
scalar/ACT, gpsimd SWDGE), split into ~128-256KB pieces ordered by first-use
time so compute never stalls on a late tensor.  res [128,2] is PE-transposed
to [2,128] so the output DMA is two contiguous 512B descriptors (a [128,x]
partition-strided store pays a multi-us receipt storm).
"""

import numpy as np
import ml_dtypes

import concourse.bass as bass
import concourse.bacc as bacc
import concourse.tile as tile
import concourse.mybir as mybir
from concourse.bass_utils import run_bass_kernel_spmd

F32 = mybir.dt.float32
BF16 = mybir.dt.bfloat16
NPBF16 = ml_dtypes.bfloat16

P = 128
NF = 127              # features incl. bias
B = 256               # batch
NPAIR = 8128          # pairs (i<=j), i,j in [0,127)
NCORES = 8
NLOC = NPAIR // NCORES          # 1016 pairs per core
NCOL = 1024                     # padded columns
NCK = 512                       # matmul chunk (PSUM fp32 bank limit)

_CACHE = {}


def _build_nc():
    nc = bacc.Bacc("TRN2", target_bir_lowering=False, debug=False)
    xt = nc.dram_tensor("xt", [P, B], BF16, kind="ExternalInput")
    wds = [nc.dram_tensor(f"wd{ck}", [P, NCK], BF16, kind="ExternalInput")
           for ck in range(2)]
    xss = [[nc.dram_tensor(f"xs{bt}c{ck}", [P, NCK], BF16, kind="ExternalInput")
            for ck in range(2)] for bt in range(2)]
    xbs = [nc.dram_tensor(f"xb{bt}", [P, NCOL], BF16, kind="ExternalInput")
           for bt in range(2)]
    ident = nc.dram_tensor("ident", [P, P], F32, kind="ExternalInput")
    out = nc.dram_tensor("out", [2, P], F32, kind="ExternalOutput")

    with tile.TileContext(nc) as tc:
        with (
            tc.tile_pool(name="const", bufs=1) as cpool,
            tc.tile_pool(name="ps", bufs=1, space="PSUM") as ps,
        ):
            xt_t = cpool.tile([P, B], BF16, tag="xt")
            wd_t = [cpool.tile([P, NCK], BF16, tag=f"wd{ck}", name=f"wd{ck}_t")
                    for ck in range(2)]
            xs_t = [[cpool.tile([P, NCK], BF16, tag=f"xs{bt}c{ck}",
                                name=f"xs{bt}c{ck}_t") for ck in range(2)]
                    for bt in range(2)]
            xb_t = [cpool.tile([P, NCOL], BF16, tag=f"xb{bt}", name=f"xb{bt}_t")
                    for bt in range(2)]
            id_t = cpool.tile([P, P], F32, tag="ident")

            # --- input DMAs: 3 channels, ordered by first-use time ---
            nc.sync.dma_start(xt_t[:], xt[:])
            nc.scalar.dma_start(wd_t[1][:], wds[1][:])
            nc.sync.dma_start(wd_t[0][:], wds[0][:])
            nc.scalar.dma_start(xs_t[0][1][:], xss[0][1][:])
            nc.gpsimd.dma_start(xs_t[0][0][:], xss[0][0][:])
            nc.sync.dma_start(xb_t[0][:], xbs[0][:])
            nc.scalar.dma_start(xs_t[1][1][:], xss[1][1][:])
            nc.gpsimd.dma_start(id_t[:], ident[:])
            nc.sync.dma_start(xs_t[1][0][:], xss[1][0][:])
            nc.scalar.dma_start(xb_t[1][:], xbs[1][:])

            res_t = cpool.tile([P, 2], F32, tag="res")
            scr_t = cpool.tile([P, NCOL], BF16, tag="scr")
            for bt in range(2):
                v1 = cpool.tile([P, NCOL], BF16, tag=f"v1_{bt}", name=f"v1_{bt}")
                # chunk c1 first: its weights (wd1) ride the scalar queue's
                # first slot, so the c1 matmul/copy complete first
                for ck in (1, 0):
                    z_ps = ps.tile([P, NCK], F32, tag=f"z{bt}{ck}",
                                   name=f"z{bt}{ck}_ps")
                    nc.tensor.matmul(
                        z_ps[:], xt_t[:, bt * P:(bt + 1) * P], wd_t[ck][:],
                        start=True, stop=True,
                    )
                    zc = cpool.tile([P, NCK], BF16, tag=f"zc{bt}{ck}",
                                    name=f"zc{bt}{ck}")
                    nc.scalar.copy(zc[:], z_ps[:])
                    nc.vector.tensor_tensor(
                        v1[:, ck * NCK:(ck + 1) * NCK],
                        zc[:],
                        xs_t[bt][ck][:],
                        mybir.AluOpType.mult,
                    )
                # fused multiply + row-accumulate via TensorScalarPtr
                nc.vector.scalar_tensor_tensor(
                    out=scr_t[:],
                    in0=v1[:],
                    scalar=1.0,
                    in1=xb_t[bt][:],
                    op0=mybir.AluOpType.mult,
                    op1=mybir.AluOpType.mult,
                    accum_out=res_t[:, bt:bt + 1],
                )

            # transpose [128,2] -> [2,128] on the PE so the output DMA is two
            # contiguous 512B descriptors instead of 128 8B ones
            tr_ps = ps.tile([2, P], F32, tag="tr")
            nc.tensor.transpose(tr_ps[:], res_t[:], id_t[:])
            stage = cpool.tile([2, P], F32, tag="stage")
            nc.vector.tensor_copy(stage[:], tr_ps[:])
            nc.sync.dma_start(out[:], stage[:])
    nc.compile()
    return nc


def _pair_maps():
    # lex-ordered pairs (i<=j): p = rsp2[i] + (j - i)
    i_of = np.repeat(np.arange(NF), NF - np.arange(NF))
    j_of = np.concatenate([np.arange(i, NF) for i in range(NF)])
    return i_of, j_of


def _prep_inputs(x, weights, comb_idx):
    """Host-side layout prep: gathers/scatters only, no arithmetic on x."""
    x = np.ascontiguousarray(np.asarray(x, dtype=np.float32))
    w = np.asarray(weights, dtype=np.float32).ravel()
    ci = np.asarray(comb_idx)
    i_, j_, k_ = (ci[:, 0].astype(np.int64), ci[:, 1].astype(np.int64),
                  ci[:, 2].astype(np.int64))

    xf = np.concatenate([np.ones((B, 1), np.float32), x], axis=1)  # [256,127]

    ar = np.arange(NF, dtype=np.int64)
    rsp2 = ar * NF - (ar * (ar - 1)) // 2
    p_of_c = rsp2[i_] + (j_ - i_)          # pair index of each triple
    W_all = np.zeros((P, NPAIR), np.float32)
    W_all[k_, p_of_c] = w

    i_of, j_of = _pair_maps()
    xf_d = xf.astype(NPBF16)
    xt = np.zeros((P, B), NPBF16)
    xt[:NF, :] = xf_d.T

    in_maps = []
    for c in range(NCORES):
        ps_ = np.arange(c, NPAIR, NCORES)
        wd = np.zeros((P, NCOL), NPBF16)
        wd[:, :NLOC] = W_all[:, ps_].astype(NPBF16)
        ic, jc = i_of[ps_], j_of[ps_]
        m = {"xt": xt, "ident": np.eye(P, dtype=np.float32)}
        for ck in range(2):
            m[f"wd{ck}"] = np.ascontiguousarray(wd[:, ck * NCK:(ck + 1) * NCK])
        for bt in range(2):
            xs = np.zeros((P, NCOL), NPBF16)
            xb = np.zeros((P, NCOL), NPBF16)
            xs[:, :NLOC] = xf_d[bt * P:(bt + 1) * P, :][:, ic]
            xb[:, :NLOC] = xf_d[bt * P:(bt + 1) * P, :][:, jc]
            for ck in range(2):
                m[f"xs{bt}c{ck}"] = np.ascontiguousarray(
                    xs[:, ck * NCK:(ck + 1) * NCK])
            m[f"xb{bt}"] = xb
        in_maps.append(m)
    return in_maps


def _get_nc():
    if "nc" not in _CACHE:
        _CACHE["nc"] = _build_nc()
    return _CACHE["nc"]


def run_spmd(x, weights, comb_idx, trace=False):
    nc = _get_nc()
    in_maps = _prep_inputs(x, weights, comb_idx)
    res = run_bass_kernel_spmd(nc, in_maps, list(range(NCORES)), trace=trace)
    acc = np.zeros(B, np.float64)
    for c in range(NCORES):
        acc += res.results[c]["out"].astype(np.float64).reshape(B)
    return acc[:, None].astype(np.float32), res


def kernel(x, weights, comb_idx):
    out, _ = run_spmd(x, weights, comb_idx, trace=False)
    return out
